# revision 1
# baseline (speedup 1.0000x reference)
"""Trainium2 Bass kernel for nn_DistancePredictor (pairwise MLP distance map).

out[b,i,j] = relu(W2 . gelu(cat(Xi,Xj,Xi-Xj,Xi*Xj) @ W1 + b1) + b2), symmetrized,
diagonal zeroed.  Decomposition used here (per row i):

    cat(...) @ W1 = X_j @ (Wp*X_i + (Wj-Wd)) + X_i @ (Wi+Wd)
                    `------- W_i (dxh) -----'   `--- A_i (bias) ---'

so each row costs one DVE op (build W_i), one 128x1024 fp32r matmul (S^T),
one gelu with per-partition bias A_i+b1, and eight 128-col x W2 matmuls that
write the output *transposed* (j on partitions) into PSUM accumulator banks.
Relu and the 0.5 symmetrize factor are folded into the evacuation (W2,b2
pre-scaled by 0.5 on host; relu commutes with positive scale).  The
symmetrize term r'[j,i] is fetched with a per-batch 8-core AllToAll of fp16
128x128 blocks (batch 0's exchange overlaps batch 1's compute), transposed
in-flight by the DMA xbar, and added on GpSimd/DVE; the diagonal mask is
per-core input data so the SPMD program is identical on all cores.
"""

import numpy as np

import concourse.bacc as bacc
import concourse.mybir as mybir
import concourse.tile as tile
from concourse.bass_utils import run_bass_kernel_spmd

F32 = mybir.dt.float32
F32R = mybir.dt.float32r
F16 = mybir.dt.float16
AF = mybir.ActivationFunctionType
ALU = mybir.AluOpType

B, L, D = 2, 1024, 128
H = 128
NCORES = 8
SLAB = L // NCORES  # 128


def build_nc(skip_collective=False, reps=1):
    nc = bacc.Bacc(
        "TRN2",
        target_bir_lowering=False,
        debug=False,
        num_devices=NCORES,
    )

    xt_in = nc.dram_tensor("xt", [B, D, L], F32R, kind="ExternalInput")
    xc_in = nc.dram_tensor("xc", [B, D, SLAB], F32, kind="ExternalInput")
    wp_in = nc.dram_tensor("wp", [D, H], F32, kind="ExternalInput")
    wb_in = nc.dram_tensor("wb", [D, H], F32, kind="ExternalInput")
    wa_in = nc.dram_tensor("wa", [D, H], F32, kind="ExternalInput")
    w2_in = nc.dram_tensor("w2h", [H, 1], F16, kind="ExternalInput")
    b1_in = nc.dram_tensor("b1c", [H, 1], F32, kind="ExternalInput")
    b2_in = nc.dram_tensor("b2c", [128, 1], F32, kind="ExternalInput")
    masks_in = nc.dram_tensor("masks", [128, NCORES * 128], F16, kind="ExternalInput")
    out_t = nc.dram_tensor("out", [B, L, SLAB], F16, kind="ExternalOutput")

    with tile.TileContext(nc) as tc:
        with (
            tc.tile_pool(name="const", bufs=1) as cp,
            tc.tile_pool(name="wpool", bufs=6) as wp_pool,
            tc.tile_pool(name="gpool", bufs=6) as g_pool,
            tc.tile_pool(name="rt", bufs=1) as rt_pool,
            tc.tile_pool(name="fin", bufs=8) as fin_pool,
            tc.tile_pool(name="ps_s", bufs=3, space="PSUM") as ps_s,
            tc.tile_pool(name="ps_acc", bufs=1, space="PSUM") as ps_acc,
            tc.tile_pool(name="dram", bufs=1, space="DRAM") as dram_pool,
        ):
            # ---- load constants / inputs to SBUF ----
            # Tensors the first row-iteration needs come first; the
            # epilogue-only masks tensor last.
            xc_sb = [cp.tile([D, SLAB], F32, name=f"xc_sb{b}") for b in range(B)]
            nc.sync.dma_start(xc_sb[0][:], xc_in[0])
            wp_sb = cp.tile([D, H], F32, name="wp_sb")
            nc.sync.dma_start(wp_sb[:], wp_in[:])
            wb_sb = cp.tile([D, H], F32, name="wb_sb")
            nc.sync.dma_start(wb_sb[:], wb_in[:])
            xt_sb = [cp.tile([D, L], F32R, name=f"xt_sb{b}") for b in range(B)]
            nc.sync.dma_start(xt_sb[0][:, 0:512], xt_in[0][:, 0:512])
            nc.gpsimd.dma_start(xt_sb[0][:, 512:1024], xt_in[0][:, 512:1024])
            w2_sb = cp.tile([H, 1], F16, name="w2_sb")
            nc.sync.dma_start(w2_sb[:], w2_in[:])
            wa_sb = cp.tile([D, H], F32, name="wa_sb")
            nc.sync.dma_start(wa_sb[:], wa_in[:])
            b1_sb = cp.tile([H, 1], F32, name="b1_sb")
            nc.sync.dma_start(b1_sb[:], b1_in[:])
            b2_sb = cp.tile([128, 1], F32, name="b2_sb")
            nc.sync.dma_start(b2_sb[:], b2_in[:])
            nc.sync.dma_start(xc_sb[1][:], xc_in[1])
            nc.sync.dma_start(xt_sb[1][:, 0:512], xt_in[1][:, 0:512])
            nc.gpsimd.dma_start(xt_sb[1][:, 512:1024], xt_in[1][:, 512:1024])
            masks_sb = cp.tile([128, NCORES * 128], F16, name="masks_sb")
            nc.sync.dma_start(masks_sb[:], masks_in[:])

            # Preload the gelu activation-table set (~2.7us) while XT streams
            # in, instead of stalling the first real gelu on it.
            warm = cp.tile([128, 1], F32, name="warm")
            nc.scalar.activation(warm[:], wp_sb[:, 0:1], AF.Gelu, bias=0.0, scale=1.0)

            # ---- A''^T = Wa^T @ Xc^T + b1 (h x SLAB per batch) ----
            at_sb = []
            for b in range(B):
                at_ps = ps_s.tile([H, SLAB], F32, tag="s")
                nc.tensor.matmul(at_ps[:], wa_sb[:], xc_sb[b][:], start=True, stop=True)
                atb = cp.tile([H, SLAB], F32, name=f"at_sb{b}")
                nc.scalar.add(atb[:], at_ps[:], add=b1_sb[:])
                at_sb.append(atb)

            # ---- A2A buffers in DRAM (per batch, so batch 0's exchange +
            # symmetrize overlap batch 1's compute) ----
            a2a_send = [
                dram_pool.tile([NCORES, SLAB, SLAB], F16, name=f"a2a_send{b}")
                for b in range(B)
            ]
            a2a_recv = [
                dram_pool.tile([NCORES, SLAB, SLAB], F16, name=f"a2a_recv{b}")
                for b in range(B)
            ]

            # ---- main loop (reps>1 only for timing experiments) ----
            rt_tiles = {}
            for rep, b in [(r, b) for r in range(reps) for b in range(B)]:
                acc0 = ps_acc.tile(
                    [128, 4 * SLAB], F32, tag="acc0", name=f"acc0_{rep}_{b}"
                )
                acc1 = ps_acc.tile(
                    [128, 4 * SLAB], F32, tag="acc1", name=f"acc1_{rep}_{b}"
                )
                accs = [acc0, acc1]
                xtr = xt_sb[b][:]
                for il in range(SLAB):
                    wtile = wp_pool.tile([D, H], F32R, tag="wi")
                    nc.vector.scalar_tensor_tensor(
                        wtile[:],
                        wp_sb[:],
                        xc_sb[b][:, il : il + 1],
                        wb_sb[:],
                        op0=ALU.mult,
                        op1=ALU.add,
                    )
                    s_ps = ps_s.tile([H, L], F32, tag="s")
                    wr = wtile[:]
                    nc.tensor.matmul(
                        s_ps[:, 0:512], wr, xtr[:, 0:512], start=True, stop=True
                    )
                    nc.tensor.matmul(
                        s_ps[:, 512:1024], wr, xtr[:, 512:1024], start=True, stop=True
                    )
                    gt = g_pool.tile([H, L], F16, tag="g")
                    nc.scalar.activation(
                        gt[:],
                        s_ps[:],
                        AF.Gelu,
                        bias=at_sb[b][:, il : il + 1],
                        scale=1.0,
                    )
                    for jt in range(NCORES):
                        q, sub = jt // 4, jt % 4
                        col = sub * SLAB + il
                        nc.tensor.matmul(
                            accs[q][:, col : col + 1],
                            gt[:, jt * 128 : (jt + 1) * 128],
                            w2_sb[:],
                            start=True,
                            stop=True,
                        )
                # evacuate accumulators: relu(x + b2/2) -> sbuf (fp16), stage
                # this q-half to the A2A send buffer in one chunked DMA.
                # On DVE (fused add+max) to keep ACT free for gelus.
                last_b = rep == reps - 1 and b == B - 1
                for q in range(2):
                    rt = rt_pool.tile([128, 4 * SLAB], F16, name=f"rt_{b}_{q}")
                    if last_b and q == 1:
                        # ACT is idle after the final gelu; run this half
                        # there so both evacuations go in parallel.
                        nc.scalar.activation(
                            rt[:], accs[q][:], AF.Relu, bias=b2_sb[:], scale=1.0
                        )
                    else:
                        nc.vector.tensor_scalar(
                            rt[:],
                            accs[q][:],
                            b2_sb[:],
                            0.0,
                            op0=ALU.add,
                            op1=ALU.max,
                        )
                    # Zero this core's diagonal block BEFORE staging: the
                    # masked values then come back from the AllToAll already
                    # masked, so no per-block mask pass is needed later.
                    mw = nc.vector if last_b else nc.gpsimd
                    mw.tensor_tensor(
                        rt[:], rt[:], masks_sb[:, q * 512 : (q + 1) * 512],
                        op=ALU.mult,
                    )
                    rt_tiles[(b, q)] = rt
                    # ACT's HWDGE queue is free once the final batch's gelus
                    # are done; before that it would stall gelus (engine FIFO)
                    stage_eng = nc.sync if q == 0 else (
                        nc.scalar if last_b else nc.gpsimd
                    )
                    stage_eng.dma_start(
                        a2a_send[b][4 * q : 4 * q + 4].rearrange("s r c -> r s c"),
                        rt[:].rearrange("r (s c) -> r s c", s=4),
                    )

                # all-to-all this batch's transposed-slab blocks
                if not skip_collective:
                    nc.gpsimd.collective_compute(
                        "AllToAll",
                        ALU.bypass,
                        replica_groups=[list(range(NCORES))],
                        ins=[a2a_send[b].opt()],
                        outs=[a2a_recv[b].opt()],
                    )

                # symmetrize: out[b, d-block, :] = own + recv^T (the diag
                # mask was already applied to rt pre-exchange; recv blocks are
                # transposed in-flight by the DMA xbar)
                for d in range(NCORES):
                    rbt = fin_pool.tile([128, 128], F16, tag="rbt")
                    if last_b and d % 2 == 0:
                        nc.scalar.dma_start_transpose(rbt[:], a2a_recv[b][d])
                    else:
                        nc.sync.dma_start_transpose(rbt[:], a2a_recv[b][d])
                    q, sub = d // 4, d % 4
                    own = rt_tiles[(b, q)][:, sub * SLAB : (sub + 1) * SLAB]
                    ob = fin_pool.tile([128, 128], F16, tag="ob")
                    # During earlier batches DVE is busy with W_i preps and a
                    # queued epilogue op would stall them (engine FIFO), so do
                    # the adds on GpSimd; on the final batch DVE is free.
                    if last_b:
                        ew = nc.vector if d % 4 != 3 else nc.gpsimd
                    else:
                        ew = nc.gpsimd
                    ew.tensor_tensor(ob[:], rbt[:], own, op=ALU.add)
                    if last_b:
                        store_eng = nc.sync if d % 2 == 1 else nc.scalar
                    else:
                        store_eng = nc.sync if d % 2 == 1 else nc.gpsimd
                    store_eng.dma_start(out_t[b, d * 128 : (d + 1) * 128, :], ob[:])

    nc.compile()
    return nc


_NC_CACHE = {}


def _get_nc():
    if "nc" not in _NC_CACHE:
        _NC_CACHE["nc"] = build_nc()
    return _NC_CACHE["nc"]


def make_in_maps(X, W1, b1, W2, b2):
    X = np.ascontiguousarray(X, dtype=np.float32)
    W1 = np.asarray(W1, dtype=np.float32)
    b1 = np.asarray(b1, dtype=np.float32)
    W2 = np.asarray(W2, dtype=np.float32)
    b2 = np.asarray(b2, dtype=np.float32)

    Wi, Wj, Wd, Wp = W1[0:128], W1[128:256], W1[256:384], W1[384:512]
    wa = np.ascontiguousarray(Wi + Wd)
    wb = np.ascontiguousarray(Wj - Wd)
    wp = np.ascontiguousarray(Wp)
    w2h = np.ascontiguousarray((0.5 * W2).astype(np.float16).reshape(H, 1))
    b1c = np.ascontiguousarray(b1.reshape(H, 1))
    b2c = np.full((128, 1), 0.5 * float(b2[0]), dtype=np.float32)
    xt = np.ascontiguousarray(X.transpose(0, 2, 1))  # (B, D, L)

    in_maps = []
    for c in range(NCORES):
        masks = np.ones((128, NCORES * 128), dtype=np.float16)
        masks[:, c * 128 : (c + 1) * 128] = (
            1.0 - np.eye(128)
        ).astype(np.float16)
        xc = np.ascontiguousarray(xt[:, :, c * SLAB : (c + 1) * SLAB])
        in_maps.append(
            {
                "xt": xt,
                "xc": xc,
                "wp": wp,
                "wb": wb,
                "wa": wa,
                "w2h": w2h,
                "b1c": b1c,
                "b2c": b2c,
                "masks": masks,
            }
        )
    return in_maps


def assemble(results):
    full = np.empty((B, L, L), dtype=np.float32)
    for c in range(NCORES):
        o = results[c]["out"]  # (B, L, SLAB) fp16: out[b, j, i_local]
        full[:, c * SLAB : (c + 1) * SLAB, :] = o.transpose(0, 2, 1).astype(
            np.float32
        )
    return full


def kernel(X, W1, b1, W2, b2, _trace=False):
    nc = _get_nc()
    in_maps = make_in_maps(X, W1, b1, W2, b2)
    res = run_bass_kernel_spmd(
        nc, in_maps, core_ids=list(range(NCORES)), trace=_trace
    )
    out = assemble(res.results)
    if _trace:
        return out, res
    return out

